# revision 67
# baseline (speedup 1.0000x reference)
"""Trainium2 Bass kernel for sparse channel-attention (XCA-style) module.

Reference computation (b=4, c=192, h=w=128, heads=6, C=32):
  qkv  = dwconv3x3(conv1x1(x, w_qkv), w_dw); ref_qkv likewise (shared weights)
  q = qkv[:, :c] (from x), k = ref_qkv[:, c:2c], v = ref_qkv[:, 2c:]
  q,k L2-normalized along tokens; attn = (q @ k^T) * temperature  [b,6,32,32]
  out = sum_i attn_w[i] * softmax(topk-threshold(attn, k_i)) @ v;  proj conv1x1.

Sharding: 8 cores = (batch 0..3) x (spatial half 0..1, 64 rows + halo).
Cross-core traffic: one 26KB AllReduce per core pair (q/k norms + q@k^T).

Device algorithm per core:
  - conv1x1 via float32r matmuls (z stored bf16, padded row stride 130)
  - dwconv3x3: q,k channels on TensorE as 9 PSUM-accumulated diag-matmuls;
    v channels on VectorE as 9-tap scalar_tensor_tensor chains
  - PE transposes q,k to token-major; Gram matmul per head-pair gives
    q@k^T blocks + q/k squared norms (diagonal)
  - AllReduce(pair) of packed stats; softmax with top-k via rank counting
    (rank_j = #{l: attn_l >= attn_j}; keep iff rank <= k_i), 4 branches
    fused into one combined A
  - final = (w_proj @ A_blockdiag) @ v  (projection folded into attention)
"""

from contextlib import ExitStack

import numpy as np
import ml_dtypes

import concourse.bass as bass
import concourse.mybir as mybir
import concourse.tile as tile
from concourse import bacc
from concourse.bass_utils import run_bass_kernel_spmd

F32 = mybir.dt.float32
F32R = mybir.dt.float32r
BF16 = mybir.dt.bfloat16
F16 = mybir.dt.float16
AL = mybir.AluOpType
AF = mybir.ActivationFunctionType

B, CDIM, H, W = 4, 192, 128, 128
HEADS, CH = 6, 32
HB = 64                      # rows per core (half image)
ROWS = HB + 2                # halo rows in z buffer (66)
ZSTRIDE = 130                # padded row stride in z (128 + 2 zero pad cols)
ZBASE = 2                    # leading guard elements in z tiles
ZLEN = ZBASE + ROWS * ZSTRIDE + 2   # 8584
ZBROWS = 34                  # zvB rows per spatial half (32 out + 2 halo)
ZBLEN = ZBASE + ZBROWS * ZSTRIDE + 2
NPXH = HB * W // 2           # pixels per vB half
NPX = HB * W                 # output pixels per core (8192)
NIN = ROWS * W               # conv input pixels per core (8448)
KS_LIST = [16, 21, 24, 25]   # top-k values for C=32
# tap order: dw=0 taps first (even parity for DVE 2x mode)
TAPS9 = [(-1, 0), (0, 0), (1, 0), (-1, -1), (-1, 1), (0, -1), (0, 1), (1, -1), (1, 1)]

_CACHE = {}
import os
TRUNC = int(os.environ.get("KTRUNC", "9"))


def _build():
    nc = bacc.Bacc("TRN2", num_devices=8, num_swdge_queues=4)

    # ---------------- kernel I/O ----------------
    x_d = nc.dram_tensor("x_sh", [CDIM, NIN], F32R, kind="ExternalInput")
    r_d = nc.dram_tensor("ref_sh", [CDIM, NIN], F32R, kind="ExternalInput")
    wq_d = nc.dram_tensor("wq_t", [CDIM, 192], F32R, kind="ExternalInput")
    wkv_d = nc.dram_tensor("wkv_t", [CDIM, 384], F32R, kind="ExternalInput")
    dqk_d = nc.dram_tensor("dqk", [3, 128, 9 * 128], F32R, kind="ExternalInput")
    vw_d = nc.dram_tensor("vw", [CDIM, 9], F32, kind="ExternalInput")
    dvB_d = nc.dram_tensor("dvB", [128, 9 * 128], BF16, kind="ExternalInput")
    qkw_d = nc.dram_tensor("qkw", [3, 128, 9], F32, kind="ExternalInput")
    dv0_d = nc.dram_tensor("dv0", [128, 9 * 128], BF16, kind="ExternalInput")
    wp_d = nc.dram_tensor("wp_t", [CDIM, 192], BF16, kind="ExternalInput")
    temp_d = nc.dram_tensor("temp_rep", [CDIM, 1], F32, kind="ExternalInput")
    aw_d = nc.dram_tensor("aw_rep", [CDIM, 4], F32, kind="ExternalInput")
    idf_d = nc.dram_tensor("ident_f32", [128, 128], F32, kind="ExternalInput")
    idb_d = nc.dram_tensor("ident_bf16", [128, 128], F16, kind="ExternalInput")
    thr_d = nc.dram_tensor("thr4", [128, 4 * 32], F32, kind="ExternalInput")
    e0_d = nc.dram_tensor("e_rep0", [HEADS, 128], F32, kind="ExternalInput")
    e1_d = nc.dram_tensor("e_rep1", [HEADS, 64], F32, kind="ExternalInput")
    out_d = nc.dram_tensor("out", [CDIM, NPX], BF16, kind="ExternalOutput")

    with tile.TileContext(nc) as tc, ExitStack() as ctx:
        consts = ctx.enter_context(tc.tile_pool(name="consts", bufs=1))
        zpool = ctx.enter_context(tc.tile_pool(name="zpool", bufs=1))
        zscp = ctx.enter_context(tc.tile_pool(name="zscp", bufs=2))
        ing = ctx.enter_context(tc.tile_pool(name="ing", bufs=2))    # input granules
        gcm = ctx.enter_context(tc.tile_pool(name="gcm", bufs=3))    # qk chan-major granules
        qktp = ctx.enter_context(tc.tile_pool(name="qktp", bufs=10))  # token-major qk tiles
        vht = ctx.enter_context(tc.tile_pool(name="vht", bufs=2))
        small = ctx.enter_context(tc.tile_pool(name="small", bufs=1))
        mps = ctx.enter_context(tc.tile_pool(name="mps", bufs=4, space="PSUM"))
        tps = ctx.enter_context(tc.tile_pool(name="tps", bufs=2, space="PSUM"))
        gpp = ctx.enter_context(tc.tile_pool(name="gpp", bufs=1, space="PSUM"))
        dram = ctx.enter_context(tc.tile_pool(name="dram", bufs=1, space="DRAM"))

        # ---------------- constant loads ----------------
        wq_sb0 = consts.tile([128, 192], F32R)
        wq_sb1 = consts.tile([64, 192], F32R)
        wkv_sb0 = consts.tile([128, 384], F32R)
        wkv_sb1 = consts.tile([64, 384], F32R)
        nc.sync.dma_start(wq_sb0[:], wq_d[0:128, :])
        nc.sync.dma_start(wq_sb1[:], wq_d[128:192, :])
        wkv_pending = [(wkv_sb0, wkv_d[0:128, :]), (wkv_sb1, wkv_d[128:192, :])]
        late_loads = []
        dqk_sb = []
        for c in range(3):
            t = consts.tile([128, 9 * 128], F32R, name=f"dqk_sb{c}")
            late_loads.append((t, dqk_d[c]))
            dqk_sb.append(t)
        vw0 = consts.tile([128, 9], F32)
        vw1 = consts.tile([64, 9], F32)
        dvB_sb = consts.tile([128, 9 * 128], BF16)
        late_loads.append((dvB_sb, dvB_d[:]))
        dv0_sb = consts.tile([128, 9 * 128], BF16)
        late_loads.append((dv0_sb, dv0_d[:]))
        qkw_sb = []
        for c in range(3):
            t = consts.tile([128, 9], F32, name=f"qkw_sb{c}")
            late_loads.append((t, qkw_d[c]))
            qkw_sb.append(t)
        late_loads.append((vw0, vw_d[0:128, :]))
        late_loads.append((vw1, vw_d[128:192, :]))
        wp0 = consts.tile([128, 192], BF16)
        wp1 = consts.tile([64, 192], BF16)
        late_loads.append((wp0, wp_d[0:128, :]))
        late_loads.append((wp1, wp_d[128:192, :]))
        temp0 = consts.tile([128, 1], F32)
        temp1 = consts.tile([64, 1], F32)
        late_loads.append((temp0, temp_d[0:128, :]))
        late_loads.append((temp1, temp_d[128:192, :]))
        aw0 = consts.tile([128, 4], F32)
        aw1 = consts.tile([64, 4], F32)
        late_loads.append((aw0, aw_d[0:128, :]))
        late_loads.append((aw1, aw_d[128:192, :]))
        ident_f32 = consts.tile([128, 128], F32)
        ident_h = consts.tile([128, 128], F16)
        late_loads.append((ident_f32, idf_d[:]))
        late_loads.append((ident_h, idb_d[:]))
        thr4 = consts.tile([128, 4 * CH], F32)
        late_loads.append((thr4, thr_d[:]))
        erep0 = consts.tile([HEADS, 128], F32)
        erep1 = consts.tile([HEADS, 64], F32)
        late_loads.append((erep0, e0_d[:]))
        late_loads.append((erep1, e1_d[:]))

        # ---------------- z buffers ----------------
        # q,k conv outputs (z) kept in f32 (bf16 z flips top-k ranks and blows
        # the error budget), held as rolling 16-row super-chunks to fit SBUF.
        # v z-buffer stays full-size bf16 (v precision barely matters).
        SC_OUT = 16
        SC_IN = SC_OUT + 2
        ZSCLEN = ZBASE + SC_IN * ZSTRIDE + 2
        zv0 = zpool.tile([128, ZLEN], BF16)
        zvB = zpool.tile([128, ZBLEN], BF16)
        v0 = zpool.tile([128, NPX], BF16)
        vB = zpool.tile([128, NPXH], BF16)
        for zt, nr in ((zv0, ROWS), (zvB, ZBROWS)):
            nc.vector.memset(zt[:, 0:ZBASE], 0.0)
            pad = zt[:, ZBASE:ZBASE + nr * ZSTRIDE].rearrange(
                "p (h w) -> p h w", w=ZSTRIDE)[:, :, 128:130]
            nc.vector.memset(pad, 0.0)

        ncopy = [0]
        # cost(free=512): ACT 612, DVE 731, Pool 806 -- ACT-heavy rotation
        COPY_ROT = ["A", "A", "D", "A"]

        def copy_any(dst, src, eng=None):
            if eng is None:
                eng = COPY_ROT[ncopy[0] % len(COPY_ROT)]
                ncopy[0] += 1
            if eng == "D":
                nc.vector.tensor_copy(dst, src)
            elif eng == "P":
                nc.gpsimd.tensor_copy(dst, src)
            else:
                nc.scalar.copy(dst, src)

        def zdst(zt, j0, nrows, p0, pw):
            # strided view of z rows j0..j0+nrows (cols 0..127)
            v = zt[p0:p0 + pw, ZBASE + ZSTRIDE * j0: ZBASE + ZSTRIDE * (j0 + nrows)]
            return v.rearrange("p (h w) -> p h w", w=ZSTRIDE)[:, :, 0:128]

        def ztap(zt, h0, nrows, dh, dw):
            # read view for output rows h0..h0+nrows, tap (dh, dw)
            start = ZBASE + ZSTRIDE * (h0 + 1 + dh) + dw
            v = zt[:, start:start + ZSTRIDE * nrows]
            return v.rearrange("p (h w) -> p h w", w=ZSTRIDE)[:, :, 0:128]

        G_ps = gpp.tile([128, 512], F32, name="G_ps")
        PC_ROWS = 4

        def emit_vB(jb):
            # v channels 128:192, packed 2 spatial halves deep: local rows
            # 4*jb..4*jb+4 in both halves at once via PE diag-matmuls
            h0l = PC_ROWS * jb
            ps = mps.tile([128, 512], F32, tag="main", name="vB_ps")
            for t, (dh, dw) in enumerate(TAPS9):
                nc.tensor.matmul(
                    ps[:, :].rearrange("p (h w) -> p h w", w=W),
                    lhsT=dvB_sb[:, t * 128:(t + 1) * 128],
                    rhs=ztap(zvB, h0l, PC_ROWS, dh, dw),
                    start=(t == 0), stop=(t == 8))
            copy_any(vB[:, h0l * W:(h0l + PC_ROWS) * W], ps[:, :])

        def emit_v0_hyb(h0g, nrows):
            # ACT computes tap products, DVE accumulates with 2x bf16 adds --
            # shifts roughly half of a v0 chain from DVE to the idle ACT
            for h1 in range(h0g, h0g + nrows, PC_ROWS):
                outv = v0[:, h1 * W:(h1 + PC_ROWS) * W].rearrange(
                    "p (h w) -> p h w", w=W)
                for t, (dh, dw) in enumerate(TAPS9):
                    iv = ztap(zv0, h1, PC_ROWS, dh, dw)
                    if t == 0:
                        nc.vector.tensor_scalar(
                            out=outv, in0=iv, scalar1=vw0[:, 0:1],
                            scalar2=None, op0=AL.mult)
                    else:
                        tmp = vht.tile([128, 512], BF16, tag="vh", name="vh_tmp")
                        tv = tmp[:].rearrange("p (h w) -> p h w", w=W)
                        nc.scalar.activation(tv, iv, AF.Copy, bias=0.0,
                                             scale=vw0[:, t:t + 1])
                        nc.vector.tensor_tensor(out=outv, in0=outv, in1=tv,
                                                op=AL.add)

        def emit_v0(h0g, nrows):
            # v channels 0:128; emitted late (after the collective) to fill
            # the collective latency window. One granule in eight goes to
            # PE-diag to balance the tail.
            if (h0g // PC_ROWS) % 8 not in (6, 7) and nrows > PC_ROWS:
                outv = v0[:, h0g * W:(h0g + nrows) * W].rearrange(
                    "p (h w) -> p h w", w=W)
                npx_ = nrows * W
                for t, (dh, dw) in enumerate(TAPS9):
                    iv = ztap(zv0, h0g, nrows, dh, dw)
                    if t == 0:
                        nc.vector.tensor_scalar(
                            out=outv, in0=iv, scalar1=vw0[:, 0:1],
                            scalar2=None, op0=AL.mult)
                    else:
                        # TS at 4x + TT at 2x beats STT (no dve perf modes)
                        tmp = vht.tile([128, 1024], BF16, tag="vh",
                                       name="vh_tmp")
                        tv = tmp[:, 0:npx_].rearrange("p (h w) -> p h w", w=W)
                        nc.vector.tensor_scalar(
                            out=tv, in0=iv, scalar1=vw0[:, t:t + 1],
                            scalar2=None, op0=AL.mult)
                        nc.vector.tensor_tensor(out=outv, in0=tv, in1=outv,
                                                op=AL.add)
            else:
                for h1 in range(h0g, h0g + nrows, PC_ROWS):
                    ps = mps.tile([128, 512], F32, tag="main", name="v0_ps")
                    for t, (dh, dw) in enumerate(TAPS9):
                        nc.tensor.matmul(
                            ps[:, :].rearrange("p (h w) -> p h w", w=W),
                            lhsT=dv0_sb[:, t * 128:(t + 1) * 128],
                            rhs=ztap(zv0, h1, PC_ROWS, dh, dw),
                            start=(t == 0), stop=(t == 8))
                    copy_any(v0[:, h1 * W:(h1 + PC_ROWS) * W], ps[:, :])

        for sc in range(4):
            # --- conv1x1 (f32r) for this super-chunk: 18 input rows ---
            zsc = []
            for c in range(3):
                t_ = zscp.tile([128, ZSCLEN], F32R, tag=f"zsc{c}", name=f"zsc{c}")
                nc.vector.memset(t_[:, 0:ZBASE].bitcast(F32), 0.0)
                padv = t_[:, ZBASE:ZBASE + SC_IN * ZSTRIDE].rearrange(
                    "p (h w) -> p h w", w=ZSTRIDE)[:, :, 128:130].bitcast(F32)
                nc.vector.memset(padv, 0.0)
                zsc.append(t_)
            for (j0, nrows) in ((0, 4), (4, 4), (8, 4), (12, 4), (16, 2)):
                xrow = SC_OUT * sc + j0
                npix = nrows * W
                n0 = xrow * W
                xg0 = ing.tile([128, 512], F32R, tag="xg0", name="xg0")
                xg1 = ing.tile([64, 512], F32R, tag="xg1", name="xg1")
                nc.sync.dma_start(xg0[:, 0:npix], x_d[0:128, n0:n0 + npix])
                nc.sync.dma_start(xg1[:, 0:npix], x_d[128:192, n0:n0 + npix])
                for (co0, cow, zi, p0) in ((0, 128, 0, 0), (128, 64, 1, 0)):
                    ps = mps.tile([128, 512], F32, tag="main", name="cv_ps")
                    nc.tensor.matmul(ps[0:cow, 0:npix],
                                     lhsT=wq_sb0[:, co0:co0 + cow],
                                     rhs=xg0[:, 0:npix], start=True, stop=False)
                    nc.tensor.matmul(ps[0:cow, 0:npix],
                                     lhsT=wq_sb1[:, co0:co0 + cow],
                                     rhs=xg1[:, 0:npix], start=False, stop=True)
                    src = ps[0:cow, 0:npix].rearrange("p (h w) -> p h w", w=W)
                    copy_any(zdst(zsc[zi], j0, nrows, p0, cow), src)
                rg0 = ing.tile([128, 512], F32R, tag="rg0", name="rg0")
                rg1 = ing.tile([64, 512], F32R, tag="rg1", name="rg1")
                nc.sync.dma_start(rg0[:, 0:npix], r_d[0:128, n0:n0 + npix])
                nc.sync.dma_start(rg1[:, 0:npix], r_d[128:192, n0:n0 + npix])
                if wkv_pending:
                    for (wt_, wd_) in wkv_pending:
                        nc.sync.dma_start(wt_[:], wd_)
                    wkv_pending = []
                # packed kv tiles: [k0:64|v128:192] -> zsc1-p64 + zv1,
                # [k64:192] -> zsc2, [v0:128] -> zv0
                kv_tiles = ((0, 128, ("pack",)), (128, 128, ("sc", 2, 0)),
                            (256, 128, ("v", zv0, 0)))
                for (co0, cow, dst) in kv_tiles:
                    ps = mps.tile([128, 512], F32, tag="main", name="cv_ps")
                    nc.tensor.matmul(ps[0:cow, 0:npix],
                                     lhsT=wkv_sb0[:, co0:co0 + cow],
                                     rhs=rg0[:, 0:npix], start=True, stop=False)
                    nc.tensor.matmul(ps[0:cow, 0:npix],
                                     lhsT=wkv_sb1[:, co0:co0 + cow],
                                     rhs=rg1[:, 0:npix], start=False, stop=True)
                    src = ps[0:cow, 0:npix].rearrange("p (h w) -> p h w", w=W)
                    if dst[0] == "sc":
                        copy_any(zdst(zsc[dst[1]], j0, nrows, dst[2], cow), src)
                    elif dst[0] == "v":
                        copy_any(zdst(dst[1], xrow, nrows, dst[2], cow), src)
                    else:
                        copy_any(zdst(zsc[1], j0, nrows, 64, 64),
                                 ps[0:64, 0:npix].rearrange(
                                     "p (h w) -> p h w", w=W))
                        a0, a1 = max(xrow, 0), min(xrow + nrows, ZBROWS)
                        if a1 > a0:
                            copy_any(zdst(zvB, a0, a1 - a0, 0, 64),
                                     ps[64:64 + 64, (a0 - xrow) * W:(a1 - xrow) * W]
                                     .rearrange("p (h w) -> p h w", w=W))
                        b0, b1 = max(xrow, 32), min(xrow + nrows, ROWS)
                        if b1 > b0:
                            copy_any(zdst(zvB, b0 - 32, b1 - b0, 64, 64),
                                     ps[64:64 + 64, (b0 - xrow) * W:(b1 - xrow) * W]
                                     .rearrange("p (h w) -> p h w", w=W))

            if sc == 0:
                for (tile_, dsrc) in late_loads:
                    nc.sync.dma_start(tile_[:], dsrc)
            # --- dwconv + transpose + Gram for output rows 16sc..16sc+16 ---
            for pcc in range(SC_OUT // PC_ROWS):
                h0l = pcc * PC_ROWS
                h0g = SC_OUT * sc + h0l
                grans = []
                pcg = sc * 4 + pcc
                for c in range(3):
                    # fp16 post-dw granules: 10 mantissa bits keep the
                    # top-k ranks stable (bf16 does not), and fp16 gets
                    # 1 cyc/row transposes + N=128 grams on PE
                    g = gcm.tile([128, 512], F16, tag=f"g{c}", name=f"gcm{c}")
                    if c == 2 and pcg % 16 < 10:
                        # DVE path: taps 0-7 accumulate in f32 scratch,
                        # tap 8 writes the fp16 granule (rounds once)
                        gsc_f = gcm.tile([128, 512], F32, tag="gf", name="gf")
                        fv = gsc_f[:].rearrange("p (h w) -> p h w", w=W)
                        gv = g[:].rearrange("p (h w) -> p h w", w=W)
                        for t, (dh, dw) in enumerate(TAPS9):
                            iv = ztap(zsc[c], h0l, PC_ROWS, dh, dw)
                            if t == 0:
                                nc.vector.tensor_scalar(
                                    out=fv, in0=iv, scalar1=qkw_sb[c][:, 0:1],
                                    scalar2=None, op0=AL.mult)
                            else:
                                nc.vector.scalar_tensor_tensor(
                                    out=(gv if t == 8 else fv), in0=iv,
                                    scalar=qkw_sb[c][:, t:t + 1],
                                    in1=fv, op0=AL.mult, op1=AL.add)
                    else:
                        ps = mps.tile([128, 512], F32, tag="main", name="dw_ps")
                        for t, (dh, dw) in enumerate(TAPS9):
                            nc.tensor.matmul(
                                ps[:, :].rearrange("p (h w) -> p h w", w=W),
                                lhsT=dqk_sb[c][:, t * 128:(t + 1) * 128],
                                rhs=ztap(zsc[c], h0l, PC_ROWS, dh, dw),
                                start=(t == 0), stop=(t == 8))
                        copy_any(g[:], ps[:])
                    grans.append(g)
                # v chunks on DVE
                if TRUNC < 4:
                    emit_v0(h0g, PC_ROWS)
                if TRUNC < 3:
                    continue
                # full 128x128 transposes only (sliced is_transpose crashes);
                # pair-grouping happens in the PSUM->SBUF copy via strided src
                qkts = []
                for r in range(PC_ROWS):
                    qkt_ps = tps.tile([128, 1024], F16, tag="tp", name="qkt_ps")
                    for c in range(3):
                        nc.tensor.transpose(
                            qkt_ps[:, 128 * c:128 * (c + 1)],
                            grans[c][:, 128 * r:128 * (r + 1)],
                            ident_h[:])
                    qkt = qktp.tile([128, 384], F16, tag="qkt", name="qkt")
                    srcv = qkt_ps[:, 0:384].rearrange(
                        "p (g hp c) -> p hp g c", g=2, hp=3)
                    copy_any(qkt[:, :].rearrange(
                        "p (hp g c) -> p hp g c", hp=3, g=2), srcv)
                    qkts.append(qkt)
                # pair-block Gram at N=128 (fp16 is 1 cyc/row at any N),
                # accumulated in one persistent PSUM bank.  start=True
                # clears the WHOLE bank, so only the global-first matmul
                # sets it; m=1/2 regions start with has_written=0 anyway.
                for m in range(3):
                    for r in range(PC_ROWS):
                        nc.tensor.matmul(
                            G_ps[:, 128 * m:128 * (m + 1)],
                            lhsT=qkts[r][:, 128 * m:128 * (m + 1)],
                            rhs=qkts[r][:, 128 * m:128 * (m + 1)],
                            start=(sc == 0 and pcc == 0 and r == 0 and m == 0),
                            stop=(sc == 3 and pcc == 3 and r == PC_ROWS - 1))

        if TRUNC < 4:
            # truncated build: dump live intermediates to out so nothing is dead
            if TRUNC >= 2:
                nc.gpsimd.dma_start(out_d[0:128, 0:NPX], v0[:, 0:NPX])
                nc.gpsimd.dma_start(out_d[128:192, 0:NPX // 2], vB[0:64, 0:NPX // 2])
            else:
                nc.gpsimd.dma_start(out_d[0:128, 0:NPX], zv0[0:128, 0:NPX])
            if TRUNC >= 3:
                gdump = small.tile([128, 384], BF16, name="gdump")
                nc.scalar.copy(gdump[:], G_ps[:, 0:384])
                nc.sync.dma_start(out_d[128:192, 0:384], gdump[0:64, :])
        if TRUNC >= 4:
            # ---------------- phase 3: stats + AllReduce ----------------
            stats_in = dram.tile([6528], F32)
            stats_out = dram.tile([2, 6528], F32)
            nrm = small.tile([128, 3], F32)
            junk = small.tile([128, 128], F32)
            pstage = small.tile([32, 6 * CH], F32, name="pstage")
            for hp in range(3):
                nc.vector.tensor_tensor(out=junk[:],
                                        in0=G_ps[:, 128 * hp:128 * (hp + 1)],
                                        in1=ident_f32[:], op=AL.mult)
                nc.vector.tensor_reduce(out=nrm[:, hp:hp + 1], in_=junk[:],
                                        axis=mybir.AxisListType.X, op=AL.add)
                for j, (r0, c0) in enumerate(((0, 64), (32, 96))):
                    head = 2 * hp + j
                    nc.scalar.copy(pstage[:, CH * head:CH * (head + 1)],
                                   G_ps[r0:r0 + 32,
                                        128 * hp + c0:128 * hp + c0 + 32])
            # two batched stats DMAs instead of 18 small ones
            nc.sync.dma_start(
                stats_in[0:6144].rearrange("(h c d) -> c h d", h=HEADS, c=CH),
                pstage[:].rearrange("p (h d) -> p h d", d=CH))
            nc.sync.dma_start(
                stats_in[6144:6336].rearrange("(hp p) -> p hp", hp=3),
                nrm[0:64, :])
            nc.sync.dma_start(
                stats_in[6336:6528].rearrange("(hp p) -> p hp", hp=3),
                nrm[64:128, :])
            if TRUNC == 35:
                nc.sync.dma_start(stats_out[0], stats_in[:])
                nc.sync.dma_start(stats_out[1], stats_in[:])
            else:
                nc.gpsimd.collective_compute(
                    "AllGather", AL.bypass,
                    replica_groups=[[0, 1], [2, 3], [4, 5], [6, 7]],
                    ins=[stats_in[:].opt()], outs=[stats_out[:].opt()])
            vplan = {0: "D", 2: "D", 4: "D", 6: "P", 7: "P",
                     8: "D", 10: "D", 12: "D", 14: "P", 15: "P"}
            for _pcg in range(HB // PC_ROWS):
                kind = vplan.get(_pcg)
                if kind == "D":
                    emit_v0(_pcg * PC_ROWS, 2 * PC_ROWS)
                elif kind == "H":
                    emit_v0_hyb(_pcg * PC_ROWS, 2 * PC_ROWS)
                elif kind == "P":
                    emit_v0(_pcg * PC_ROWS, PC_ROWS)
                if _pcg % 2 == 0 and _pcg // 2 not in (6, 7):
                    emit_vB(_pcg // 2)

            warm = small.tile([128, 1], F32, name="act_warm")
            nc.vector.memset(warm[:], 1.0)
            nc.scalar.sqrt(warm[:], warm[:])
            nc.scalar.activation(warm[:], warm[:], AF.Exp, bias=0.0, scale=1.0)

            if TRUNC in (4, 35):
                so_dbg = small.tile([128, 51], F32, name='so_dbg')
                nc.sync.dma_start(so_dbg[:], stats_out[0].rearrange('(p d) -> p d', d=51))
                nc.gpsimd.dma_start(out_d[0:128, 0:51], so_dbg[:])
            if TRUNC >= 5 and TRUNC != 35:
                # ---------------- phase 4: softmax with top-k rank masks ----------------
                att2 = [small.tile([128, 2 * CH], F32, name="att2_0"),
                        small.tile([64, 2 * CH], F32, name="att2_1")]
                nc.sync.dma_start(
                    att2[0][:].rearrange("p (g d) -> p g d", g=2),
                    stats_out[:, 0:4096].rearrange("g (p d) -> p g d", d=CH))
                nc.scalar.dma_start(
                    att2[1][:].rearrange("p (g d) -> p g d", g=2),
                    stats_out[:, 4096:6144].rearrange("g (p d) -> p g d", d=CH))
                qsq2 = [small.tile([128, 2], F32, name="qsq2_0"),
                        small.tile([64, 2], F32, name="qsq2_1")]
                nc.sync.dma_start(
                    qsq2[0][:], stats_out[:, 6144:6272].rearrange("g p -> p g"))
                nc.scalar.dma_start(
                    qsq2[1][:], stats_out[:, 6272:6336].rearrange("g p -> p g"))
                ksql2 = small.tile([HEADS, 2 * CH], F32)
                nc.sync.dma_start(
                    ksql2[:].rearrange("h (g d) -> h g d", g=2),
                    stats_out[:, 6336:6528].rearrange("g (h d) -> h g d", d=CH))
                attn = [small.tile([128, CH], F32, name="attn0"),
                        small.tile([64, CH], F32, name="attn1")]
                for i in range(2):
                    nc.vector.tensor_tensor(out=attn[i][:], in0=att2[i][:, 0:CH],
                                            in1=att2[i][:, CH:2 * CH], op=AL.add)
                ksql = small.tile([HEADS, CH], F32)
                nc.vector.tensor_tensor(out=ksql[:], in0=ksql2[:, 0:CH],
                                        in1=ksql2[:, CH:2 * CH], op=AL.add)
                # replicate ksq across rows of each head via tiny matmul
                # kq tiles hold [ksqr | qsq]; krep writes ksqr directly into
                # them, skipping a separate staging copy
                kqt = [small.tile([128, CH + 1], F32, name="kq0"),
                       small.tile([64, CH + 1], F32, name="kq1")]
                for i, (erep, pw) in enumerate(((erep0, 128), (erep1, 64))):
                    ps = mps.tile([128, 512], F32, tag="main", name="krep_ps")
                    nc.tensor.matmul(ps[0:pw, 0:CH], lhsT=erep[:], rhs=ksql[:],
                                     start=True, stop=True)
                    nc.vector.tensor_copy(kqt[i][:, 0:CH], ps[0:pw, 0:CH])
                # held-back PE v-work fills the PE-idle softmax window
                emit_vB(6)
                emit_vB(7)

                abd0 = small.tile([128, 192], BF16)
                abd1 = small.tile([64, 192], BF16)
                nc.vector.memset(abd0[:], 0.0)
                nc.vector.memset(abd1[:], 0.0)

                for ti, pw in ((0, 128), (1, 64)):
                    at = attn[ti]
                    tempt = temp0 if ti == 0 else temp1
                    awt = aw0 if ti == 0 else aw1
                    # kq = [ksqr | qsq]; one sqrt + one reciprocal covers both
                    kq = kqt[ti]
                    nc.vector.tensor_tensor(out=kq[0:pw, CH:CH + 1],
                                            in0=qsq2[ti][:, 0:1],
                                            in1=qsq2[ti][:, 1:2], op=AL.add)
                    rt = small.tile([128, CH + 1], F32, tag="rt", name="rt")
                    nc.scalar.sqrt(rt[0:pw, :], kq[0:pw, :])
                    nc.vector.reciprocal(rt[0:pw, :], rt[0:pw, :])
                    s_c = small.tile([128, 1], F32, tag="s", name="s_c")
                    nc.vector.tensor_mul(s_c[0:pw, :], rt[0:pw, CH:CH + 1], tempt[:])
                    # attn = (P * s_c) * inv_nk in one fused op
                    nc.vector.scalar_tensor_tensor(out=at[:], in0=at[:],
                                                   scalar=s_c[0:pw, 0:1],
                                                   in1=rt[0:pw, 0:CH],
                                                   op0=AL.mult, op1=AL.mult)
                    # ranks
                    cmp = small.tile([128, CH * CH], F32, tag="cmp", name="cmp")
                    c3 = cmp[0:pw, :].rearrange("p (j k) -> p j k", k=CH)
                    nc.vector.tensor_tensor(
                        out=c3, in0=at[:, None, :].broadcast_to([pw, CH, CH]),
                        in1=at[:, :, None].broadcast_to([pw, CH, CH]), op=AL.is_ge)
                    rk = small.tile([128, CH], F32, tag="rk", name="rk")
                    nc.vector.tensor_reduce(out=rk[0:pw, :], in_=c3,
                                            axis=mybir.AxisListType.X, op=AL.add)
                    # e = exp(attn - rowmax)
                    nmax = small.tile([128, 1], F32, tag="nm", name="nmax")
                    nc.vector.tensor_reduce(out=nmax[0:pw, :], in_=at[:],
                                            axis=mybir.AxisListType.X, op=AL.max,
                                            negate=True)
                    ex = small.tile([128, CH], F32, tag="ex", name="ex")
                    nc.scalar.activation(ex[0:pw, :], at[:], AF.Exp,
                                         bias=nmax[0:pw, 0:1], scale=1.0)
                    # all 4 top-k branches batched along the free axis
                    mk4 = small.tile([128, 4 * CH], F32, tag="mk4", name="mk4")
                    m3 = mk4[0:pw, :].rearrange("p (i j) -> p i j", j=CH)
                    nc.vector.tensor_tensor(
                        out=m3, in0=rk[0:pw, None, :].broadcast_to([pw, 4, CH]),
                        in1=thr4[0:pw, :].rearrange("p (i j) -> p i j", j=CH),
                        op=AL.is_le)
                    ej4 = small.tile([128, 4 * CH], F32, tag="ej4", name="ej4")
                    e3 = ej4[0:pw, :].rearrange("p (i j) -> p i j", j=CH)
                    nc.vector.tensor_tensor(
                        out=e3, in0=ex[0:pw, None, :].broadcast_to([pw, 4, CH]),
                        in1=m3, op=AL.mult)
                    sden = small.tile([128, 4], F32, tag="sd", name="sden")
                    nc.vector.tensor_reduce(out=sden[0:pw, :], in_=e3,
                                            axis=mybir.AxisListType.X, op=AL.add)
                    coef = small.tile([128, 4], F32, tag="cf", name="coef")
                    nc.vector.reciprocal(coef[0:pw, :], sden[0:pw, :])
                    nc.vector.tensor_mul(coef[0:pw, :], coef[0:pw, :], awt[:])
                    # gt = sum_i coef[:, i] * mk4[:, i, :]
                    gt = small.tile([128, CH], F32, tag="gt", name="gt")
                    for i in range(4):
                        if i == 0:
                            nc.vector.tensor_scalar(out=gt[0:pw, :],
                                                    in0=mk4[0:pw, 0:CH],
                                                    scalar1=coef[0:pw, 0:1],
                                                    scalar2=None, op0=AL.mult)
                        else:
                            nc.vector.scalar_tensor_tensor(
                                out=gt[0:pw, :], in0=mk4[0:pw, CH * i:CH * (i + 1)],
                                scalar=coef[0:pw, i:i + 1], in1=gt[0:pw, :],
                                op0=AL.mult, op1=AL.add)
                    # A blocks into block-diagonal abd (bf16)
                    abdt = abd0 if ti == 0 else abd1
                    nheads_t = 4 if ti == 0 else 2
                    for j in range(nheads_t):
                        head = j if ti == 0 else 4 + j
                        nc.vector.tensor_tensor(
                            out=abdt[32 * j:32 * (j + 1), 32 * head:32 * (head + 1)],
                            in0=ex[32 * j:32 * (j + 1), :],
                            in1=gt[32 * j:32 * (j + 1), :], op=AL.mult)

                if TRUNC == 5:
                    nc.gpsimd.dma_start(out_d[0:128, 0:192], abd0[:])
                    nc.gpsimd.dma_start(out_d[128:192, 0:192], abd1[:])
                if TRUNC >= 9:
                    # ---------------- phase 5: M2^T = A_bd^T @ Wp^T, final = M2 @ v ----------------
                    m2tb = [small.tile([128, 192], BF16, name="m2tb0"),
                            small.tile([128, 192], BF16, name="m2tb1")]
                    for dt_i, (d0, dw_) in enumerate(((0, 128), (128, 64))):
                        ps = mps.tile([128, 512], F32, tag="main", name="m2_ps")
                        nc.tensor.matmul(ps[0:dw_, 0:192], lhsT=abd0[:, d0:d0 + dw_],
                                         rhs=wp0[:], start=True, stop=False)
                        nc.tensor.matmul(ps[0:dw_, 0:192], lhsT=abd1[:, d0:d0 + dw_],
                                         rhs=wp1[:], start=False, stop=True)
                        if dt_i == 0:
                            nc.scalar.copy(m2tb[0][:], ps[0:128, 0:192])
                        else:
                            nc.scalar.copy(m2tb[1][0:64, :], ps[0:64, 0:192])
                            nc.scalar.copy(m2tb[1][64:128, :], ps[0:64, 0:192])

                    FCH = 512
                    for ct, (co0, cw) in enumerate(((0, 128), (128, 64))):
                        fo = None
                        for fc in range(NPX // FCH):
                            n0 = fc * FCH
                            half = fc // 8
                            nB0 = (fc % 8) * FCH
                            ps = mps.tile([128, 512], F32, tag="main", name="fo_ps")
                            nc.tensor.matmul(ps[0:cw, :], lhsT=m2tb[0][:, co0:co0 + cw],
                                             rhs=v0[:, n0:n0 + FCH], start=True, stop=False)
                            nc.tensor.matmul(
                                ps[0:cw, :],
                                lhsT=m2tb[1][64 * half:64 * half + 64, co0:co0 + cw],
                                rhs=vB[64 * half:64 * half + 64, nB0:nB0 + FCH],
                                start=False, stop=True)
                            if fc % 2 == 0:
                                fo = ing.tile([128, 2 * FCH], BF16, tag="fo",
                                              name="fo_sb", bufs=6)
                            copy_any(fo[0:cw, FCH * (fc % 2):FCH * (fc % 2 + 1)],
                                     ps[0:cw, :], eng=["A", "D"][fc % 2])
                            if fc % 2 == 1:
                                nc.sync.dma_start(
                                    out_d[co0:co0 + cw, n0 - FCH:n0 + FCH],
                                    fo[0:cw, :])

    nc.finalize()
    return nc


def _prep_inputs(x, ref, w_qkv, w_dw, w_proj, temperature, attn_w):
    bf = ml_dtypes.bfloat16
    w_qkv = np.asarray(w_qkv, np.float32)[:, :, 0, 0]          # [576, 192]
    w_dw = np.asarray(w_dw, np.float32)[:, 0]                  # [576, 3, 3]
    w_proj = np.asarray(w_proj, np.float32)[:, :, 0, 0]        # [192, 192]
    temp = np.asarray(temperature, np.float32).reshape(HEADS)
    aw = np.asarray(attn_w, np.float32).reshape(4)

    wq_t = np.ascontiguousarray(w_qkv[:192].T)                 # [ci, co]
    wkv_raw = w_qkv[192:].T                                    # [ci, 384]
    # packed kv output-tile order: [k0:64 | v128:192 | k64:192 | v0:128]
    wkv_t = np.ascontiguousarray(np.concatenate(
        [wkv_raw[:, 0:64], wkv_raw[:, 320:384],
         wkv_raw[:, 64:192], wkv_raw[:, 192:320]], axis=1))

    dwq, dwk, dwv = w_dw[:192], w_dw[192:384], w_dw[384:]
    chunks = [dwq[0:128], np.concatenate([dwq[128:192], dwk[0:64]]), dwk[64:192]]
    dqk = np.zeros((3, 128, 9 * 128), np.float32)
    for c, blk in enumerate(chunks):
        for t, (dh, dw_) in enumerate(TAPS9):
            np.fill_diagonal(dqk[c, :, t * 128:(t + 1) * 128], blk[:, dh + 1, dw_ + 1])
    qkw = np.zeros((3, 128, 9), np.float32)
    for c, blk in enumerate(chunks):
        for t, (dh, dw_) in enumerate(TAPS9):
            qkw[c, :, t] = blk[:, dh + 1, dw_ + 1]
    vw = np.zeros((CDIM, 9), np.float32)
    for t, (dh, dw_) in enumerate(TAPS9):
        vw[:, t] = dwv[:, dh + 1, dw_ + 1]
    dvB = np.zeros((128, 9 * 128), np.float32)
    for t, (dh, dw_) in enumerate(TAPS9):
        np.fill_diagonal(dvB[:, t * 128:(t + 1) * 128],
                         np.tile(dwv[128:192, dh + 1, dw_ + 1], 2))
    dv0 = np.zeros((128, 9 * 128), np.float32)
    for t, (dh, dw_) in enumerate(TAPS9):
        np.fill_diagonal(dv0[:, t * 128:(t + 1) * 128], dwv[0:128, dh + 1, dw_ + 1])

    wp_t = np.ascontiguousarray(w_proj.T)                      # [c, co]
    temp_rep = np.repeat(temp, CH).reshape(CDIM, 1)
    aw_rep = np.tile(aw[None, :], (CDIM, 1))
    ident = np.eye(128, dtype=np.float32)
    e0 = np.zeros((HEADS, 128), np.float32)
    e1 = np.zeros((HEADS, 64), np.float32)
    for h in range(4):
        e0[h, 32 * h:32 * (h + 1)] = 1.0
    for h in range(2):
        e1[h + 4, 32 * h:32 * (h + 1)] = 1.0

    xp = np.zeros((B, CDIM, H + 2, W), np.float32)
    xp[:, :, 1:H + 1] = np.asarray(x, np.float32)
    rp = np.zeros((B, CDIM, H + 2, W), np.float32)
    rp[:, :, 1:H + 1] = np.asarray(ref, np.float32)

    thr4 = np.zeros((128, 4 * CH), np.float32)
    for i, kv in enumerate(KS_LIST):
        thr4[:, CH * i:CH * (i + 1)] = float(kv)

    common = {
        "wq_t": wq_t, "wkv_t": wkv_t, "thr4": thr4,
        "dqk": dqk, "vw": vw, "qkw": qkw, "dvB": dvB.astype(bf), "dv0": dv0.astype(bf),
        "wp_t": wp_t.astype(bf), "temp_rep": temp_rep, "aw_rep": aw_rep,
        "ident_f32": ident, "ident_bf16": ident.astype(np.float16),
        "e_rep0": e0, "e_rep1": e1,
    }
    in_maps = []
    for core in range(8):
        b, s = core // 2, core % 2
        m = dict(common)
        m["x_sh"] = np.ascontiguousarray(
            xp[b, :, 64 * s:64 * s + ROWS].reshape(CDIM, NIN))
        m["ref_sh"] = np.ascontiguousarray(
            rp[b, :, 64 * s:64 * s + ROWS].reshape(CDIM, NIN))
        in_maps.append(m)
    return in_maps


def _run(inputs, trace=False):
    if "nc" not in _CACHE:
        _CACHE["nc"] = _build()
    nc = _CACHE["nc"]
    in_maps = _prep_inputs(**inputs)
    res = run_bass_kernel_spmd(nc, in_maps, core_ids=list(range(8)), trace=trace)
    out = np.zeros((B, CDIM, H, W), np.float32)
    for core in range(8):
        b, s = core // 2, core % 2
        out[b, :, 64 * s:64 * (s + 1)] = np.asarray(
            res.results[core]["out"], dtype=np.float32).reshape(CDIM, HB, W)
    return out, res


def kernel(**inputs):
    out, _ = _run(inputs, trace=False)
    return out

